# revision 15
# baseline (speedup 1.0000x reference)
"""Trainium2 Bass kernel for nn_MemoryGraphBackprop (GNN message passing).

Strategy (v3)
-------------
T=64 sequential steps over state [BS=2, N=1024, D=64] on ONE NeuronCore
(the recurrence is latency-bound; an 8-core shard would need a per-step
all-gather that dwarfs the compute).  Everything SBUF-resident.

Math per step t:
    r   = A @ pm  (+ cc_t into nodes < C)
    g_t = decay * (1 - eot[b,t])          # per-partition in layout-2
    h'  = g_t*h + (1-g_t)*r               # h-state (NOT u=prim*h)
    pm' = tanh(prim * h')

v3 structure:
  - fp8(e4m3) DoubleRow matmuls: A and pm quantized to fp8; each matmul
    instruction contracts 2 src chunks (K=256) at 2x bf16 FLOP rate.
    Layout-2 psum r: [128 part = b*64+d, dst node free], 2 halves of 512.
  - h-state chain eliminates the per-step w2=(1-dt)*prim tensor:
      sb  = g_t*h (+ host-precomputed (1-g)*cc for nodes < C)   [DVE]
      h'  = (ps * (1-g_t)) + sb   -- one fused scalar_tensor_tensor/half
      u'  = h' * prim             -- half0 on GPSIMD, half1 on DVE
    then PE transposes u' quarters to layout-1, ACT fuses tanh into the
    PSUM->SBUF copy producing fp8 pm' (next step's stationary) and the
    fp32 output slice.
  - cc inject, eot gates all host-precomputed into [128,T]-indexed consts,
    so the device code is branch-free and input-agnostic (uniform decay).

Layouts:
  l2 (state h, psum r):  [128 part = b*64+d, 1024 free = n]
  l1 (pm, matmul lhsT):  [128 part = n%128, free = (n//128, b*64+d)]
"""

import os
import sys

if "/opt/trn_rl_repo" not in sys.path:
    sys.path.insert(0, "/opt/trn_rl_repo")

import numpy as np

import concourse.bass as bass
import concourse.mybir as mybir
import concourse.tile as tile
from concourse import bass_utils

BS, T, C, D = 2, 64, 64, 64
N = 1024
NT = N // 128  # 8 node chunks
P = 128        # BS*D partitions in layout-2

F32 = mybir.dt.float32
BF16 = mybir.dt.bfloat16
F8 = mybir.dt.float8e4
DR = mybir.MatmulPerfMode.DoubleRow
MULT = mybir.AluOpType.mult
ADD = mybir.AluOpType.add

# ---------------------------------------------------------------------------
# Workaround: this container's walrus accepts only ONE sync-wait per
# instruction.  (1) Tile's tail drain attaches one wait per live semaphore —
# split across multiple drains.  (2) Any multi-wait instruction gets its
# extra waits hoisted onto InstEventSemaphore carriers just before it.
# ---------------------------------------------------------------------------
from concourse.vector_clock import ScopedClock  # noqa: E402

SIM_MODE = False  # True: skip walrus-only workarounds so CoreSim can run
_ORIG_DRAIN = tile.TileContext._drain_and_barrier


def _patched_drain_and_barrier(self, tick_clock, wait_clock):
    if SIM_MODE:
        return _ORIG_DRAIN(self, tick_clock, wait_clock)
    drain_inst = self.nc.sync.drain()
    wait_clock.add_sem_waits(
        drain_inst.ins, ScopedClock({None: tick_clock.global_clock})
    )
    si = drain_inst.ins.sync_info
    if si is not None and si.on_wait is not None and len(si.on_wait) > 1:
        waits = list(si.on_wait)
        drain_inst.ins.sync_info = mybir.SyncInfo(
            on_wait=[waits[0]], on_update=si.on_update
        )
        for w in waits[1:]:
            d2 = self.nc.sync.drain()
            d2.ins.sync_info = mybir.SyncInfo(on_wait=[w], on_update=[])

    self.nc.all_engine_barrier()
    assert self.sems is not None
    popped = self.nc._tile_sem_poison_stack.pop()
    assert popped is self._sem_poison
    self.nc.clear_and_free_semaphores(list(self.sems.allocated().values()))
    self.nc.all_engine_barrier()


tile.TileContext._drain_and_barrier = _patched_drain_and_barrier


def _split_multi_waits(nc):
    if SIM_MODE:
        return 0
    n_carriers = 0
    for bb in nc.m.functions[0].blocks:
        insts = list(bb.instructions)
        out = []
        changed = False
        for inst in insts:
            si = inst.sync_info
            if si is not None and si.on_wait is not None and len(si.on_wait) > 1:
                waits = list(si.on_wait)
                for w in waits[:-1]:
                    n_carriers += 1
                    carrier = mybir.InstEventSemaphore(
                        name=f"waitsplit-{n_carriers}", ins=[], outs=[]
                    )
                    carrier.engine = inst.engine
                    carrier.sync_info = mybir.SyncInfo(on_wait=[w], on_update=[])
                    out.append(carrier)
                inst.sync_info = mybir.SyncInfo(
                    on_wait=[waits[-1]], on_update=si.on_update
                )
                changed = True
            out.append(inst)
        if changed:
            bb.instructions = out
    return n_carriers


# ---------------------------------------------------------------------------
# Host-side input massaging (layouts, scatter into dense A, norms, sigmoid).
# ---------------------------------------------------------------------------
def _prep_host(inputs):
    import ml_dtypes

    bf16 = ml_dtypes.bfloat16
    f8 = ml_dtypes.float8_e4m3

    cc = np.asarray(inputs["cc_signals"], dtype=np.float32)       # [B,T,C,D]
    eot = np.asarray(inputs["eot_mask"]).astype(bool)             # [B,T]
    idx = np.asarray(inputs["conn_indices"]).astype(np.int64)     # [N,K]
    cmask = np.asarray(inputs["conn_mask"]).astype(np.float32)    # [N,K]
    prim = np.asarray(inputs["primitives"], dtype=np.float32)     # [N,D]
    w = np.asarray(inputs["conn_weights"], dtype=np.float32)      # [N,K]
    dlog = np.asarray(inputs["decay_logit"], dtype=np.float32)    # [N]
    h0 = np.asarray(inputs["h0"], dtype=np.float32)               # [B,N,D]
    pm0 = np.asarray(inputs["prev_msg0"], dtype=np.float32)       # [B,N,D]

    # dense adjacency, transposed for the layout-2 matmul (rhs[m, n] = A[n, m])
    A = np.zeros((N, N), dtype=np.float32)
    np.add.at(A, (np.arange(N)[:, None], idx), w * cmask)
    At = np.ascontiguousarray(A.T)                                # [m, n]
    at_host = At.reshape(NT, 128, N).transpose(1, 0, 2).reshape(128, NT * N)

    # L2-normalized cc, layout-2: [b*64+d partitions, t*64 + n(<C) free]
    nrm = np.maximum(np.linalg.norm(cc, axis=-1, keepdims=True), 1e-8)
    ccn = (cc / nrm).astype(np.float32)
    cc2_host = np.ascontiguousarray(
        ccn.transpose(0, 3, 1, 2).reshape(P, T * C)
    )

    decay = (1.0 / (1.0 + np.exp(-dlog.astype(np.float64)))).astype(np.float32)
    uniform = bool(np.all(decay == decay[0]))

    prim_l2 = np.ascontiguousarray(np.tile(prim.T, (BS, 1)))      # [128, N]
    fmat = np.repeat((~eot).astype(np.float32), D, axis=0)        # [128, T]

    h0_l2 = h0.transpose(0, 2, 1).reshape(P, N)                   # [b*64+d, n]

    pm0_l1 = np.ascontiguousarray(
        pm0.reshape(BS, NT, 128, D).transpose(2, 1, 0, 3).reshape(128, NT * P)
    )

    host = {
        "at": at_host.astype(f8),
        "prim": prim_l2.astype(bf16),
        "pm0": pm0_l1.astype(f8),
    }
    if uniform:
        g = decay[0] * fmat                                       # [128, T]
        h1g = 1.0 - g
        host["gmat"] = np.ascontiguousarray(g.astype(np.float32))
        host["h1g"] = np.ascontiguousarray(h1g.astype(np.float32))
        host["h0"] = np.ascontiguousarray(h0_l2.astype(bf16))
        # (1-g)-scaled cc inject, indexed [p, t*C + c]
        ccw = cc2_host * np.repeat(h1g, C, axis=1)
        host["ccw"] = np.ascontiguousarray(ccw.astype(bf16))
    else:
        u0 = np.ascontiguousarray(prim_l2 * h0_l2)
        host["u0"] = u0.astype(bf16)
        host["cc2"] = cc2_host.astype(bf16)
        dec_l2 = np.ascontiguousarray(np.broadcast_to(decay[None, :], (P, N)))
        host["dec"] = dec_l2.astype(bf16)
        host["dp"] = (prim_l2 * decay[None, :]).astype(bf16)
        host["fmat"] = np.ascontiguousarray(fmat.astype(np.float32))
    return host, uniform


# ---------------------------------------------------------------------------
# Device kernel (v3, uniform-decay fast path)
# ---------------------------------------------------------------------------
def _build_bass_v3():
    nc = bass.Bass("TRN2", target_bir_lowering=False, debug=False)

    at_d = nc.dram_tensor("at", [128, NT * N], F8, kind="ExternalInput")
    prim_d = nc.dram_tensor("prim", [P, N], BF16, kind="ExternalInput")
    h0_d = nc.dram_tensor("h0", [P, N], BF16, kind="ExternalInput")
    pm0_d = nc.dram_tensor("pm0", [128, NT * P], F8, kind="ExternalInput")
    g_d = nc.dram_tensor("gmat", [P, T], F32, kind="ExternalInput")
    h1g_d = nc.dram_tensor("h1g", [P, T], F32, kind="ExternalInput")
    ccw_d = nc.dram_tensor("ccw", [P, T * C], BF16, kind="ExternalInput")
    out_d = nc.dram_tensor("out", [T, C, P], F32, kind="ExternalOutput")

    Tanh = mybir.ActivationFunctionType.Tanh
    STEPS = int(os.environ.get("KSTEPS", T))

    with tile.TileContext(nc) as tc:
        with (
            tc.tile_pool(name="consts", bufs=1) as consts,
            tc.tile_pool(name="state", bufs=3) as state,
            tc.tile_pool(name="tmp", bufs=3) as tmp,
            tc.tile_pool(name="psr", bufs=2, space="PSUM") as psr,
            tc.tile_pool(name="ptp", bufs=2, space="PSUM") as ptp,
        ):
            id128_sb = consts.tile([128, 128], BF16)
            from concourse.masks import make_identity
            make_identity(nc, id128_sb[:])

            # HAM warm-up: dummy matmuls keep the PE activity monitor at
            # full clock while the input DMAs land.
            warm_ps = psr.tile([128, 128], F32, tag="ps0", name="warm_ps")
            for i in range(64):
                nc.tensor.matmul(
                    warm_ps[:], id128_sb[:], id128_sb[:],
                    start=(i == 0), stop=(i == 63), skip_group_check=True,
                )

            # --- state + consts loads (state first so step-0 deps clear) ---
            h = state.tile([P, N], BF16, tag="h", name="h")
            pm = [
                state.tile([128, 4, 128], F8, tag="pm0h", name="pm_lo"),
                state.tile([128, 4, 128], F8, tag="pm1h", name="pm_hi"),
            ]
            nc.sync.dma_start(out=h[:], in_=h0_d.ap()[:])
            for hh in range(2):
                nc.sync.dma_start(
                    out=pm[hh][:], in_=pm0_d.ap()[:, hh * 512:(hh + 1) * 512]
                )
            at_sb = consts.tile([128, NT, N], F8)
            for hh in range(2):
                for m in range(NT):
                    nc.sync.dma_start(
                        out=at_sb[:, m, hh * 512:(hh + 1) * 512],
                        in_=at_d.ap()[:, m * N + hh * 512:m * N + (hh + 1) * 512],
                    )
            prim_sb = consts.tile([P, N], BF16)
            nc.sync.dma_start(out=prim_sb[:], in_=prim_d.ap()[:])
            g_sb = consts.tile([P, T], F32)
            nc.sync.dma_start(out=g_sb[:], in_=g_d.ap()[:])
            h1g_sb = consts.tile([P, T], F32)
            nc.sync.dma_start(out=h1g_sb[:], in_=h1g_d.ap()[:])
            ccw_sb = consts.tile([P, T * C], BF16)
            for q in range(4):
                s = slice(q * (T * C) // 4, (q + 1) * (T * C) // 4)
                nc.sync.dma_start(out=ccw_sb[:, s], in_=ccw_d.ap()[:, s])

            for t in range(STEPS):
                last = t == T - 1
                gt = g_sb[:, t:t + 1]
                ngt = h1g_sb[:, t:t + 1]

                # ---- sb = g*h (+ (1-g)*cc on nodes < C); off critical path
                sb = tmp.tile([P, N], BF16, tag="sb")
                nc.vector.scalar_tensor_tensor(
                    sb[:, 0:C], h[:, 0:C], gt,
                    ccw_sb[:, t * C:(t + 1) * C], MULT, ADD,
                )
                nc.vector.tensor_scalar_mul(sb[:, C:512], h[:, C:512], gt)
                if not last:
                    nc.vector.tensor_scalar_mul(sb[:, 512:N], h[:, 512:N], gt)

                # ---- matmuls: r = A @ pm (fp8 DoubleRow), layout-2 psum ----
                ps = [
                    psr.tile([P, 512], F32, tag="ps0", name="ps0"),
                    psr.tile([P, 512], F32, tag="ps1", name="ps1"),
                ]
                for hh in range(2):
                    if last and hh == 1:
                        break  # last step: only chunk 0 reaches the output
                    for jp in range(4):  # src-chunk pairs (2jp, 2jp+1)
                        nc.tensor.matmul(
                            ps[hh][:],
                            pm[jp // 2][:, (jp % 2) * 2:(jp % 2) * 2 + 2, :],
                            at_sb[:, 2 * jp:2 * jp + 2, hh * 512:(hh + 1) * 512],
                            start=(jp == 0),
                            stop=(jp == 3),
                            perf_mode=DR,
                            skip_group_check=True,
                        )

                # ---- chain: h' = ps*(1-g) + sb ; u' = h'*prim ----
                hn = state.tile([P, N], BF16, tag="h", name="hn")
                un = tmp.tile([P, N], BF16, tag="un")
                pmn = [
                    state.tile([128, 4, 128], F8, tag="pm0h", name="pmn_lo"),
                    state.tile([128, 4, 128], F8, tag="pm1h", name="pmn_hi"),
                ]
                pts = [
                    ptp.tile([128, 256], BF16, tag="pt", name="pt")
                    for _ in range(4)
                ]
                out_sb = tmp.tile([C, P], F32, tag="out_sb")

                if last:
                    nc.vector.scalar_tensor_tensor(
                        hn[:, 0:128], ps[0][:, 0:128], ngt, sb[:, 0:128],
                        MULT, ADD,
                    )
                    nc.vector.tensor_mul(
                        un[:, 0:128], hn[:, 0:128], prim_sb[:, 0:128]
                    )
                    nc.tensor.transpose(
                        pts[0][:, 0:128], un[:, 0:128], id128_sb[:]
                    )
                    nc.scalar.activation(out_sb[:], pts[0][0:C, 0:P], Tanh)
                    nc.sync.dma_start(out=out_d.ap()[t], in_=out_sb[:])
                    break

                for hh in range(2):
                    hsl = slice(hh * 512, (hh + 1) * 512)
                    nc.vector.scalar_tensor_tensor(
                        hn[:, hsl], ps[hh][:], ngt, sb[:, hsl], MULT, ADD,
                    )
                    # u' = h'*prim: half0 on GPSIMD (SBUF-only engine),
                    # half1 on DVE so the tail chains in-order with no sem.
                    if hh == 0:
                        nc.gpsimd.tensor_mul(
                            un[:, hsl], hn[:, hsl], prim_sb[:, hsl]
                        )
                    else:
                        nc.vector.tensor_mul(
                            un[:, hsl], hn[:, hsl], prim_sb[:, hsl]
                        )
                    for hq in range(2):
                        q = hh * 2 + hq
                        for j in range(2):
                            ch = q * 2 + j
                            nc.tensor.transpose(
                                pts[q][:, j * 128:(j + 1) * 128],
                                un[:, ch * 128:(ch + 1) * 128],
                                id128_sb[:],
                            )
                        nc.scalar.activation(
                            pmn[q // 2][:, (q % 2) * 2:(q % 2) * 2 + 2, :],
                            pts[q][:], Tanh,
                        )

                # fp32 output slice (nodes < C live in transposed chunk 0)
                nc.scalar.activation(out_sb[:], pts[0][0:C, 0:P], Tanh)
                nc.sync.dma_start(out=out_d.ap()[t], in_=out_sb[:])

                h, pm = hn, pmn

    _split_multi_waits(nc)
    return nc


# ---------------------------------------------------------------------------
# Fallback kernel (non-uniform decay): previous-generation structure.
# ---------------------------------------------------------------------------
def _build_bass_fallback():
    nc = bass.Bass("TRN2", target_bir_lowering=False, debug=False)

    at_d = nc.dram_tensor("at", [128, NT * N], F8, kind="ExternalInput")
    cc2_d = nc.dram_tensor("cc2", [P, T * C], BF16, kind="ExternalInput")
    prim_d = nc.dram_tensor("prim", [P, N], BF16, kind="ExternalInput")
    u0_d = nc.dram_tensor("u0", [P, N], BF16, kind="ExternalInput")
    pm0_d = nc.dram_tensor("pm0", [128, NT * P], F8, kind="ExternalInput")
    out_d = nc.dram_tensor("out", [T, C, P], F32, kind="ExternalOutput")
    dec_d = nc.dram_tensor("dec", [P, N], BF16, kind="ExternalInput")
    dp_d = nc.dram_tensor("dp", [P, N], BF16, kind="ExternalInput")
    f_d = nc.dram_tensor("fmat", [P, T], F32, kind="ExternalInput")

    Tanh = mybir.ActivationFunctionType.Tanh

    with tile.TileContext(nc) as tc:
        with (
            tc.tile_pool(name="consts", bufs=1) as consts,
            tc.tile_pool(name="state", bufs=3) as state,
            tc.tile_pool(name="tmp", bufs=3) as tmp,
            tc.tile_pool(name="psr", bufs=3, space="PSUM") as psr,
            tc.tile_pool(name="ptp", bufs=2, space="PSUM") as ptp,
        ):
            id128_sb = consts.tile([128, 128], BF16)
            from concourse.masks import make_identity
            make_identity(nc, id128_sb[:])

            warm_ps = psr.tile([128, 128], F32, tag="ps0", name="warm_ps")
            for i in range(64):
                nc.tensor.matmul(
                    warm_ps[:], id128_sb[:], id128_sb[:],
                    start=(i == 0), stop=(i == 63), skip_group_check=True,
                )

            u = [
                state.tile([P, 512], BF16, tag="u0h", name="u_lo"),
                state.tile([P, 512], BF16, tag="u1h", name="u_hi"),
            ]
            pm = [
                state.tile([128, 4, 128], F8, tag="pm0h", name="pm_lo"),
                state.tile([128, 4, 128], F8, tag="pm1h", name="pm_hi"),
            ]
            for hh in range(2):
                nc.sync.dma_start(out=u[hh][:], in_=u0_d.ap()[:, hh * 512:(hh + 1) * 512])
                nc.sync.dma_start(out=pm[hh][:], in_=pm0_d.ap()[:, hh * 512:(hh + 1) * 512])
            at_sb = consts.tile([128, NT, N], F8)
            for hh in range(2):
                for m in range(NT):
                    nc.sync.dma_start(
                        out=at_sb[:, m, hh * 512:(hh + 1) * 512],
                        in_=at_d.ap()[:, m * N + hh * 512:m * N + (hh + 1) * 512],
                    )
            prim_sb = consts.tile([P, N], BF16)
            nc.sync.dma_start(out=prim_sb[:], in_=prim_d.ap()[:])
            dec_sb = consts.tile([P, N], BF16)
            nc.sync.dma_start(out=dec_sb[:], in_=dec_d.ap()[:])
            dp_sb = consts.tile([P, N], BF16)
            nc.sync.dma_start(out=dp_sb[:], in_=dp_d.ap()[:])
            f_sb = consts.tile([P, T], F32)
            nc.sync.dma_start(out=f_sb[:], in_=f_d.ap()[:])
            cc2_sb = consts.tile([P, T * C], BF16)
            for q in range(4):
                s = slice(q * (T * C) // 4, (q + 1) * (T * C) // 4)
                nc.sync.dma_start(out=cc2_sb[:, s], in_=cc2_d.ap()[:, s])

            for t in range(T):
                sb_t = [
                    tmp.tile([P, 512], BF16, tag="sb0", name="sb_lo"),
                    tmp.tile([P, 512], BF16, tag="sb1", name="sb_hi"),
                ]
                w2 = tmp.tile([P, N], BF16, tag="w2")
                ft = f_sb[:, t:t + 1]
                w0 = tmp.tile([P, N], BF16, tag="w0")
                nc.vector.tensor_scalar_mul(w0[:], dec_sb[:], ft)
                nc.vector.tensor_mul(sb_t[0][:], u[0][:], w0[:, 0:512])
                nc.vector.tensor_mul(sb_t[1][:], u[1][:], w0[:, 512:1024])
                w1 = tmp.tile([P, N], BF16, tag="w1")
                nc.vector.tensor_scalar_mul(w1[:], dp_sb[:], ft)
                nc.vector.tensor_sub(w2[:], prim_sb[:], w1[:])
                cw = tmp.tile([P, C], BF16, tag="cw")
                nc.vector.tensor_mul(
                    cw[:], w2[:, 0:C], cc2_sb[:, t * C:(t + 1) * C]
                )
                nc.vector.tensor_add(sb_t[0][:, 0:C], sb_t[0][:, 0:C], cw[:])

                ps = [
                    psr.tile([P, 512], F32, tag="ps0", name="ps0"),
                    psr.tile([P, 512], F32, tag="ps1", name="ps1"),
                ]
                un = [
                    state.tile([P, 512], BF16, tag="u0h", name="un_lo"),
                    state.tile([P, 512], BF16, tag="u1h", name="un_hi"),
                ]
                pmn = [
                    state.tile([128, 4, 128], F8, tag="pm0h", name="pmn_lo"),
                    state.tile([128, 4, 128], F8, tag="pm1h", name="pmn_hi"),
                ]
                pts = [
                    ptp.tile([128, 256], BF16, tag="pt", name="pt")
                    for _ in range(4)
                ]
                for hh in range(2):
                    for jp in range(4):
                        nc.tensor.matmul(
                            ps[hh][:],
                            pm[jp // 2][:, (jp % 2) * 2:(jp % 2) * 2 + 2, :],
                            at_sb[:, 2 * jp:2 * jp + 2, hh * 512:(hh + 1) * 512],
                            start=(jp == 0),
                            stop=(jp == 3),
                            perf_mode=DR,
                            skip_group_check=True,
                        )
                    for hq in range(2):
                        q = hh * 2 + hq
                        if t == T - 1 and q > 0:
                            continue
                        psl = slice(hq * 256, (hq + 1) * 256)
                        x = tmp.tile([P, 256], BF16, tag=f"x{q}", name="x")
                        nc.vector.tensor_mul(
                            x[:], ps[hh][:, psl], w2[:, q * 256:(q + 1) * 256]
                        )
                        nc.vector.tensor_add(
                            un[hh][:, psl], x[:], sb_t[hh][:, psl]
                        )

                out_sb = tmp.tile([C, P], F32, tag="out_sb")
                for q in range(4):
                    if t == T - 1 and q > 0:
                        continue
                    hh, hq = divmod(q, 2)
                    for j in range(2):
                        if t == T - 1 and (hq * 2 + j) > 0:
                            continue
                        mloc = hq * 2 + j
                        nc.tensor.transpose(
                            pts[q][:, j * 128:(j + 1) * 128],
                            un[hh][:, mloc * 128:(mloc + 1) * 128],
                            id128_sb[:],
                        )
                    if t < T - 1:
                        nc.scalar.activation(
                            pmn[q // 2][:, (q % 2) * 2:(q % 2) * 2 + 2, :],
                            pts[q][:], Tanh,
                        )
                    if q == 0:
                        nc.scalar.activation(out_sb[:], pts[0][0:C, 0:P], Tanh)
                nc.sync.dma_start(out=out_d.ap()[t], in_=out_sb[:])

                u, pm = un, pmn

    _split_multi_waits(nc)
    return nc


RUN_KWARGS: dict = {}
_BUILT: dict = {}


def _get_built(uniform):
    if uniform not in _BUILT:
        _BUILT[uniform] = _build_bass_v3() if uniform else _build_bass_fallback()
    return _BUILT[uniform]


def kernel(**inputs) -> np.ndarray:
    host, uniform = _prep_host(inputs)
    nc = _get_built(uniform)
    res = bass_utils.run_bass_kernel_spmd(nc, [host], core_ids=[0], **RUN_KWARGS)
    kernel.last_result = res
    out_dev = res.results[0]["out"]                               # [T, C, 128]
    out = out_dev.reshape(T, C, BS, D).transpose(2, 0, 1, 3)      # [B,T,C,D]
    return np.ascontiguousarray(out)


if __name__ == "__main__":
    print("standalone smoke: building bass module (uniform decay path)...")
    _get_built(True)
    print("built ok")


# revision 17
# speedup vs baseline: 1.1410x; 1.1410x over previous
"""Trainium2 Bass kernel for nn_MemoryGraphBackprop (GNN message passing).

Strategy (v3)
-------------
T=64 sequential steps over state [BS=2, N=1024, D=64] on ONE NeuronCore
(the recurrence is latency-bound; an 8-core shard would need a per-step
all-gather that dwarfs the compute).  Everything SBUF-resident.

Math per step t:
    r   = A @ pm  (+ cc_t into nodes < C)
    g_t = decay * (1 - eot[b,t])          # per-partition in layout-2
    h'  = g_t*h + (1-g_t)*r               # h-state (NOT u=prim*h)
    pm' = tanh(prim * h')

v3 structure:
  - fp8(e4m3) DoubleRow matmuls: A and pm quantized to fp8; each matmul
    instruction contracts 2 src chunks (K=256) at 2x bf16 FLOP rate.
    Layout-2 psum r: [128 part = b*64+d, dst node free], 2 halves of 512.
  - h-state chain eliminates the per-step w2=(1-dt)*prim tensor:
      sb  = g_t*h (+ host-precomputed (1-g)*cc for nodes < C)   [DVE]
      h'  = (ps * (1-g_t)) + sb   -- one fused scalar_tensor_tensor/half
      u'  = h' * prim             -- half0 on GPSIMD, half1 on DVE
    then PE transposes u' quarters to layout-1, ACT fuses tanh into the
    PSUM->SBUF copy producing fp8 pm' (next step's stationary) and the
    fp32 output slice.
  - cc inject, eot gates all host-precomputed into [128,T]-indexed consts,
    so the device code is branch-free and input-agnostic (uniform decay).

Layouts:
  l2 (state h, psum r):  [128 part = b*64+d, 1024 free = n]
  l1 (pm, matmul lhsT):  [128 part = n%128, free = (n//128, b*64+d)]
"""

import os
import sys

if "/opt/trn_rl_repo" not in sys.path:
    sys.path.insert(0, "/opt/trn_rl_repo")

import numpy as np

import concourse.bass as bass
import concourse.mybir as mybir
import concourse.tile as tile
from concourse import bass_utils

BS, T, C, D = 2, 64, 64, 64
N = 1024
NT = N // 128  # 8 node chunks
P = 128        # BS*D partitions in layout-2

F32 = mybir.dt.float32
BF16 = mybir.dt.bfloat16
F8 = mybir.dt.float8e4
DR = mybir.MatmulPerfMode.DoubleRow
MULT = mybir.AluOpType.mult
ADD = mybir.AluOpType.add

# ---------------------------------------------------------------------------
# Workaround: this container's walrus accepts only ONE sync-wait per
# instruction.  (1) Tile's tail drain attaches one wait per live semaphore —
# split across multiple drains.  (2) Any multi-wait instruction gets its
# extra waits hoisted onto InstEventSemaphore carriers just before it.
# ---------------------------------------------------------------------------
from concourse.vector_clock import ScopedClock  # noqa: E402

SIM_MODE = False  # True: skip walrus-only workarounds so CoreSim can run
_ORIG_DRAIN = tile.TileContext._drain_and_barrier


def _patched_drain_and_barrier(self, tick_clock, wait_clock):
    if SIM_MODE:
        return _ORIG_DRAIN(self, tick_clock, wait_clock)
    drain_inst = self.nc.sync.drain()
    wait_clock.add_sem_waits(
        drain_inst.ins, ScopedClock({None: tick_clock.global_clock})
    )
    si = drain_inst.ins.sync_info
    if si is not None and si.on_wait is not None and len(si.on_wait) > 1:
        waits = list(si.on_wait)
        drain_inst.ins.sync_info = mybir.SyncInfo(
            on_wait=[waits[0]], on_update=si.on_update
        )
        for w in waits[1:]:
            d2 = self.nc.sync.drain()
            d2.ins.sync_info = mybir.SyncInfo(on_wait=[w], on_update=[])

    self.nc.all_engine_barrier()
    assert self.sems is not None
    popped = self.nc._tile_sem_poison_stack.pop()
    assert popped is self._sem_poison
    self.nc.clear_and_free_semaphores(list(self.sems.allocated().values()))
    self.nc.all_engine_barrier()


tile.TileContext._drain_and_barrier = _patched_drain_and_barrier


def _split_multi_waits(nc):
    if SIM_MODE:
        return 0
    n_carriers = 0
    for bb in nc.m.functions[0].blocks:
        insts = list(bb.instructions)
        out = []
        changed = False
        for inst in insts:
            si = inst.sync_info
            if si is not None and si.on_wait is not None and len(si.on_wait) > 1:
                waits = list(si.on_wait)
                for w in waits[:-1]:
                    n_carriers += 1
                    carrier = mybir.InstEventSemaphore(
                        name=f"waitsplit-{n_carriers}", ins=[], outs=[]
                    )
                    carrier.engine = inst.engine
                    carrier.sync_info = mybir.SyncInfo(on_wait=[w], on_update=[])
                    out.append(carrier)
                inst.sync_info = mybir.SyncInfo(
                    on_wait=[waits[-1]], on_update=si.on_update
                )
                changed = True
            out.append(inst)
        if changed:
            bb.instructions = out
    return n_carriers


# ---------------------------------------------------------------------------
# Host-side input massaging (layouts, scatter into dense A, norms, sigmoid).
# ---------------------------------------------------------------------------
def _prep_host(inputs):
    import ml_dtypes

    bf16 = ml_dtypes.bfloat16
    f8 = ml_dtypes.float8_e4m3

    cc = np.asarray(inputs["cc_signals"], dtype=np.float32)       # [B,T,C,D]
    eot = np.asarray(inputs["eot_mask"]).astype(bool)             # [B,T]
    idx = np.asarray(inputs["conn_indices"]).astype(np.int64)     # [N,K]
    cmask = np.asarray(inputs["conn_mask"]).astype(np.float32)    # [N,K]
    prim = np.asarray(inputs["primitives"], dtype=np.float32)     # [N,D]
    w = np.asarray(inputs["conn_weights"], dtype=np.float32)      # [N,K]
    dlog = np.asarray(inputs["decay_logit"], dtype=np.float32)    # [N]
    h0 = np.asarray(inputs["h0"], dtype=np.float32)               # [B,N,D]
    pm0 = np.asarray(inputs["prev_msg0"], dtype=np.float32)       # [B,N,D]

    # dense adjacency, transposed for the layout-2 matmul (rhs[m, n] = A[n, m])
    A = np.zeros((N, N), dtype=np.float32)
    np.add.at(A, (np.arange(N)[:, None], idx), w * cmask)
    At = np.ascontiguousarray(A.T)                                # [m, n]
    at_host = At.reshape(NT, 128, N).transpose(1, 0, 2).reshape(128, NT * N)

    # L2-normalized cc, layout-2: [b*64+d partitions, t*64 + n(<C) free]
    nrm = np.maximum(np.linalg.norm(cc, axis=-1, keepdims=True), 1e-8)
    ccn = (cc / nrm).astype(np.float32)
    cc2_host = np.ascontiguousarray(
        ccn.transpose(0, 3, 1, 2).reshape(P, T * C)
    )

    decay = (1.0 / (1.0 + np.exp(-dlog.astype(np.float64)))).astype(np.float32)
    uniform = bool(np.all(decay == decay[0]))

    prim_l2 = np.ascontiguousarray(np.tile(prim.T, (BS, 1)))      # [128, N]
    fmat = np.repeat((~eot).astype(np.float32), D, axis=0)        # [128, T]

    h0_l2 = h0.transpose(0, 2, 1).reshape(P, N)                   # [b*64+d, n]

    pm0_l1 = np.ascontiguousarray(
        pm0.reshape(BS, NT, 128, D).transpose(2, 1, 0, 3).reshape(128, NT * P)
    )

    host = {
        "at": at_host.astype(f8),
        "prim": prim_l2.astype(bf16),
        "pm0": pm0_l1.astype(f8),
    }
    if uniform:
        g = decay[0] * fmat                                       # [128, T]
        h1g = 1.0 - g
        host["gmat"] = np.ascontiguousarray(g.astype(np.float32))
        host["h1g"] = np.ascontiguousarray(h1g.astype(np.float32))
        host["h0"] = np.ascontiguousarray(h0_l2.astype(bf16))
        # (1-g)-scaled cc inject, indexed [p, t*C + c]
        ccw = cc2_host * np.repeat(h1g, C, axis=1)
        host["ccw"] = np.ascontiguousarray(ccw.astype(bf16))
    else:
        u0 = np.ascontiguousarray(prim_l2 * h0_l2)
        host["u0"] = u0.astype(bf16)
        host["cc2"] = cc2_host.astype(bf16)
        dec_l2 = np.ascontiguousarray(np.broadcast_to(decay[None, :], (P, N)))
        host["dec"] = dec_l2.astype(bf16)
        host["dp"] = (prim_l2 * decay[None, :]).astype(bf16)
        host["fmat"] = np.ascontiguousarray(fmat.astype(np.float32))
    return host, uniform


# ---------------------------------------------------------------------------
# Device kernel (v3, uniform-decay fast path)
# ---------------------------------------------------------------------------
def _build_bass_v3():
    nc = bass.Bass("TRN2", target_bir_lowering=False, debug=False)

    at_d = nc.dram_tensor("at", [128, NT * N], F8, kind="ExternalInput")
    prim_d = nc.dram_tensor("prim", [P, N], BF16, kind="ExternalInput")
    h0_d = nc.dram_tensor("h0", [P, N], BF16, kind="ExternalInput")
    pm0_d = nc.dram_tensor("pm0", [128, NT * P], F8, kind="ExternalInput")
    g_d = nc.dram_tensor("gmat", [P, T], F32, kind="ExternalInput")
    h1g_d = nc.dram_tensor("h1g", [P, T], F32, kind="ExternalInput")
    ccw_d = nc.dram_tensor("ccw", [P, T * C], BF16, kind="ExternalInput")
    out_d = nc.dram_tensor("out", [T, C, P], F32, kind="ExternalOutput")

    Tanh = mybir.ActivationFunctionType.Tanh
    STEPS = int(os.environ.get("KSTEPS", T))

    with tile.TileContext(nc) as tc:
        with (
            tc.tile_pool(name="consts", bufs=1) as consts,
            tc.tile_pool(name="state", bufs=3) as state,
            tc.tile_pool(name="tmp", bufs=3) as tmp,
            tc.tile_pool(name="psr", bufs=3, space="PSUM") as psr,
            tc.tile_pool(name="ptp", bufs=2, space="PSUM") as ptp,
        ):
            id128_sb = consts.tile([128, 128], BF16)
            from concourse.masks import make_identity
            make_identity(nc, id128_sb[:])

            # HAM warm-up: dummy matmuls keep the PE activity monitor at
            # full clock while the input DMAs land.
            warm_ps = psr.tile([128, 128], F32, tag="ps0", name="warm_ps")
            for i in range(64):
                nc.tensor.matmul(
                    warm_ps[:], id128_sb[:], id128_sb[:],
                    start=(i == 0), stop=(i == 63), skip_group_check=True,
                )

            # --- state + consts loads (state first so step-0 deps clear) ---
            h = state.tile([P, N], BF16, tag="h", name="h")
            pm = [
                state.tile([128, 4, 128], F8, tag="pm0h", name="pm_lo"),
                state.tile([128, 4, 128], F8, tag="pm1h", name="pm_hi"),
            ]
            nc.sync.dma_start(out=h[:], in_=h0_d.ap()[:])
            for hh in range(2):
                nc.sync.dma_start(
                    out=pm[hh][:], in_=pm0_d.ap()[:, hh * 512:(hh + 1) * 512]
                )
            at_sb = consts.tile([128, NT, N], F8)
            for hh in range(2):
                for m in range(NT):
                    nc.sync.dma_start(
                        out=at_sb[:, m, hh * 512:(hh + 1) * 512],
                        in_=at_d.ap()[:, m * N + hh * 512:m * N + (hh + 1) * 512],
                    )
            prim_sb = consts.tile([P, N], BF16)
            nc.sync.dma_start(out=prim_sb[:], in_=prim_d.ap()[:])
            g_sb = consts.tile([P, T], F32)
            nc.sync.dma_start(out=g_sb[:], in_=g_d.ap()[:])
            h1g_sb = consts.tile([P, T], F32)
            nc.sync.dma_start(out=h1g_sb[:], in_=h1g_d.ap()[:])
            ccw_sb = consts.tile([P, T * C], BF16)
            for q in range(4):
                s = slice(q * (T * C) // 4, (q + 1) * (T * C) // 4)
                nc.sync.dma_start(out=ccw_sb[:, s], in_=ccw_d.ap()[:, s])

            for t in range(STEPS):
                last = t == T - 1
                gt = g_sb[:, t:t + 1]
                ngt = h1g_sb[:, t:t + 1]

                # ---- sb = g*h (+ (1-g)*cc on nodes < C); off critical path
                sb = tmp.tile([P, N], BF16, tag="sb")
                nc.vector.scalar_tensor_tensor(
                    sb[:, 0:C], h[:, 0:C], gt,
                    ccw_sb[:, t * C:(t + 1) * C], MULT, ADD,
                )
                nc.vector.tensor_scalar_mul(sb[:, C:512], h[:, C:512], gt)
                if not last:
                    nc.vector.tensor_scalar_mul(sb[:, 512:N], h[:, 512:N], gt)

                # ---- matmuls: r = A @ pm (fp8 DoubleRow), layout-2 psum ----
                ps = [
                    psr.tile([P, 512], F32, tag="ps0", name="ps0"),
                    psr.tile([P, 512], F32, tag="ps1", name="ps1"),
                ]
                for hh in range(2):
                    if last and hh == 1:
                        break  # last step: only chunk 0 reaches the output
                    for jp in range(4):  # src-chunk pairs (2jp, 2jp+1)
                        nc.tensor.matmul(
                            ps[hh][:],
                            pm[jp // 2][:, (jp % 2) * 2:(jp % 2) * 2 + 2, :],
                            at_sb[:, 2 * jp:2 * jp + 2, hh * 512:(hh + 1) * 512],
                            start=(jp == 0),
                            stop=(jp == 3),
                            perf_mode=DR,
                            skip_group_check=True,
                        )

                # ---- chain: h' = ps*(1-g) + sb ; u' = h'*prim ----
                hn = state.tile([P, N], BF16, tag="h", name="hn")
                un = tmp.tile([P, N], BF16, tag="un")
                pmn = [
                    state.tile([128, 4, 128], F8, tag="pm0h", name="pmn_lo"),
                    state.tile([128, 4, 128], F8, tag="pm1h", name="pmn_hi"),
                ]
                pts = [
                    ptp.tile([128, 256], BF16, tag="pt", name="pt")
                    for _ in range(4)
                ]
                out_sb = tmp.tile([C, P], F32, tag="out_sb")

                if last:
                    nc.vector.scalar_tensor_tensor(
                        hn[:, 0:128], ps[0][:, 0:128], ngt, sb[:, 0:128],
                        MULT, ADD,
                    )
                    nc.vector.tensor_mul(
                        un[:, 0:128], hn[:, 0:128], prim_sb[:, 0:128]
                    )
                    nc.tensor.transpose(
                        pts[0][:, 0:128], un[:, 0:128], id128_sb[:]
                    )
                    nc.scalar.activation(out_sb[:], pts[0][0:C, 0:P], Tanh)
                    nc.sync.dma_start(out=out_d.ap()[t], in_=out_sb[:])
                    break

                for hh in range(2):
                    hsl = slice(hh * 512, (hh + 1) * 512)
                    nc.vector.scalar_tensor_tensor(
                        hn[:, hsl], ps[hh][:], ngt, sb[:, hsl], MULT, ADD,
                    )
                    # u' = h'*prim: GPSIMD is slow (~2.25ns/elem) — give it
                    # only chunks 0-1; DVE (2x mode) takes the rest in-order.
                    if hh == 0:
                        nc.gpsimd.tensor_mul(
                            un[:, 0:256], hn[:, 0:256], prim_sb[:, 0:256]
                        )
                        nc.vector.tensor_mul(
                            un[:, 256:512], hn[:, 256:512], prim_sb[:, 256:512]
                        )
                    else:
                        nc.vector.tensor_mul(
                            un[:, hsl], hn[:, hsl], prim_sb[:, hsl]
                        )
                    for hq in range(2):
                        q = hh * 2 + hq
                        for j in range(2):
                            ch = q * 2 + j
                            nc.tensor.transpose(
                                pts[q][:, j * 128:(j + 1) * 128],
                                un[:, ch * 128:(ch + 1) * 128],
                                id128_sb[:],
                            )
                        nc.scalar.activation(
                            pmn[q // 2][:, (q % 2) * 2:(q % 2) * 2 + 2, :],
                            pts[q][:], Tanh,
                        )

                # fp32 output slice (nodes < C live in transposed chunk 0)
                nc.scalar.activation(out_sb[:], pts[0][0:C, 0:P], Tanh)
                nc.sync.dma_start(out=out_d.ap()[t], in_=out_sb[:])

                h, pm = hn, pmn

    _split_multi_waits(nc)
    return nc


# ---------------------------------------------------------------------------
# Fallback kernel (non-uniform decay): previous-generation structure.
# ---------------------------------------------------------------------------
def _build_bass_fallback():
    nc = bass.Bass("TRN2", target_bir_lowering=False, debug=False)

    at_d = nc.dram_tensor("at", [128, NT * N], F8, kind="ExternalInput")
    cc2_d = nc.dram_tensor("cc2", [P, T * C], BF16, kind="ExternalInput")
    prim_d = nc.dram_tensor("prim", [P, N], BF16, kind="ExternalInput")
    u0_d = nc.dram_tensor("u0", [P, N], BF16, kind="ExternalInput")
    pm0_d = nc.dram_tensor("pm0", [128, NT * P], F8, kind="ExternalInput")
    out_d = nc.dram_tensor("out", [T, C, P], F32, kind="ExternalOutput")
    dec_d = nc.dram_tensor("dec", [P, N], BF16, kind="ExternalInput")
    dp_d = nc.dram_tensor("dp", [P, N], BF16, kind="ExternalInput")
    f_d = nc.dram_tensor("fmat", [P, T], F32, kind="ExternalInput")

    Tanh = mybir.ActivationFunctionType.Tanh

    with tile.TileContext(nc) as tc:
        with (
            tc.tile_pool(name="consts", bufs=1) as consts,
            tc.tile_pool(name="state", bufs=3) as state,
            tc.tile_pool(name="tmp", bufs=3) as tmp,
            tc.tile_pool(name="psr", bufs=3, space="PSUM") as psr,
            tc.tile_pool(name="ptp", bufs=2, space="PSUM") as ptp,
        ):
            id128_sb = consts.tile([128, 128], BF16)
            from concourse.masks import make_identity
            make_identity(nc, id128_sb[:])

            warm_ps = psr.tile([128, 128], F32, tag="ps0", name="warm_ps")
            for i in range(64):
                nc.tensor.matmul(
                    warm_ps[:], id128_sb[:], id128_sb[:],
                    start=(i == 0), stop=(i == 63), skip_group_check=True,
                )

            u = [
                state.tile([P, 512], BF16, tag="u0h", name="u_lo"),
                state.tile([P, 512], BF16, tag="u1h", name="u_hi"),
            ]
            pm = [
                state.tile([128, 4, 128], F8, tag="pm0h", name="pm_lo"),
                state.tile([128, 4, 128], F8, tag="pm1h", name="pm_hi"),
            ]
            for hh in range(2):
                nc.sync.dma_start(out=u[hh][:], in_=u0_d.ap()[:, hh * 512:(hh + 1) * 512])
                nc.sync.dma_start(out=pm[hh][:], in_=pm0_d.ap()[:, hh * 512:(hh + 1) * 512])
            at_sb = consts.tile([128, NT, N], F8)
            for hh in range(2):
                for m in range(NT):
                    nc.sync.dma_start(
                        out=at_sb[:, m, hh * 512:(hh + 1) * 512],
                        in_=at_d.ap()[:, m * N + hh * 512:m * N + (hh + 1) * 512],
                    )
            prim_sb = consts.tile([P, N], BF16)
            nc.sync.dma_start(out=prim_sb[:], in_=prim_d.ap()[:])
            dec_sb = consts.tile([P, N], BF16)
            nc.sync.dma_start(out=dec_sb[:], in_=dec_d.ap()[:])
            dp_sb = consts.tile([P, N], BF16)
            nc.sync.dma_start(out=dp_sb[:], in_=dp_d.ap()[:])
            f_sb = consts.tile([P, T], F32)
            nc.sync.dma_start(out=f_sb[:], in_=f_d.ap()[:])
            cc2_sb = consts.tile([P, T * C], BF16)
            for q in range(4):
                s = slice(q * (T * C) // 4, (q + 1) * (T * C) // 4)
                nc.sync.dma_start(out=cc2_sb[:, s], in_=cc2_d.ap()[:, s])

            for t in range(T):
                sb_t = [
                    tmp.tile([P, 512], BF16, tag="sb0", name="sb_lo"),
                    tmp.tile([P, 512], BF16, tag="sb1", name="sb_hi"),
                ]
                w2 = tmp.tile([P, N], BF16, tag="w2")
                ft = f_sb[:, t:t + 1]
                w0 = tmp.tile([P, N], BF16, tag="w0")
                nc.vector.tensor_scalar_mul(w0[:], dec_sb[:], ft)
                nc.vector.tensor_mul(sb_t[0][:], u[0][:], w0[:, 0:512])
                nc.vector.tensor_mul(sb_t[1][:], u[1][:], w0[:, 512:1024])
                w1 = tmp.tile([P, N], BF16, tag="w1")
                nc.vector.tensor_scalar_mul(w1[:], dp_sb[:], ft)
                nc.vector.tensor_sub(w2[:], prim_sb[:], w1[:])
                cw = tmp.tile([P, C], BF16, tag="cw")
                nc.vector.tensor_mul(
                    cw[:], w2[:, 0:C], cc2_sb[:, t * C:(t + 1) * C]
                )
                nc.vector.tensor_add(sb_t[0][:, 0:C], sb_t[0][:, 0:C], cw[:])

                ps = [
                    psr.tile([P, 512], F32, tag="ps0", name="ps0"),
                    psr.tile([P, 512], F32, tag="ps1", name="ps1"),
                ]
                un = [
                    state.tile([P, 512], BF16, tag="u0h", name="un_lo"),
                    state.tile([P, 512], BF16, tag="u1h", name="un_hi"),
                ]
                pmn = [
                    state.tile([128, 4, 128], F8, tag="pm0h", name="pmn_lo"),
                    state.tile([128, 4, 128], F8, tag="pm1h", name="pmn_hi"),
                ]
                pts = [
                    ptp.tile([128, 256], BF16, tag="pt", name="pt")
                    for _ in range(4)
                ]
                for hh in range(2):
                    for jp in range(4):
                        nc.tensor.matmul(
                            ps[hh][:],
                            pm[jp // 2][:, (jp % 2) * 2:(jp % 2) * 2 + 2, :],
                            at_sb[:, 2 * jp:2 * jp + 2, hh * 512:(hh + 1) * 512],
                            start=(jp == 0),
                            stop=(jp == 3),
                            perf_mode=DR,
                            skip_group_check=True,
                        )
                    for hq in range(2):
                        q = hh * 2 + hq
                        if t == T - 1 and q > 0:
                            continue
                        psl = slice(hq * 256, (hq + 1) * 256)
                        x = tmp.tile([P, 256], BF16, tag=f"x{q}", name="x")
                        nc.vector.tensor_mul(
                            x[:], ps[hh][:, psl], w2[:, q * 256:(q + 1) * 256]
                        )
                        nc.vector.tensor_add(
                            un[hh][:, psl], x[:], sb_t[hh][:, psl]
                        )

                out_sb = tmp.tile([C, P], F32, tag="out_sb")
                for q in range(4):
                    if t == T - 1 and q > 0:
                        continue
                    hh, hq = divmod(q, 2)
                    for j in range(2):
                        if t == T - 1 and (hq * 2 + j) > 0:
                            continue
                        mloc = hq * 2 + j
                        nc.tensor.transpose(
                            pts[q][:, j * 128:(j + 1) * 128],
                            un[hh][:, mloc * 128:(mloc + 1) * 128],
                            id128_sb[:],
                        )
                    if t < T - 1:
                        nc.scalar.activation(
                            pmn[q // 2][:, (q % 2) * 2:(q % 2) * 2 + 2, :],
                            pts[q][:], Tanh,
                        )
                    if q == 0:
                        nc.scalar.activation(out_sb[:], pts[0][0:C, 0:P], Tanh)
                nc.sync.dma_start(out=out_d.ap()[t], in_=out_sb[:])

                u, pm = un, pmn

    _split_multi_waits(nc)
    return nc


RUN_KWARGS: dict = {}
_BUILT: dict = {}


def _get_built(uniform):
    if uniform not in _BUILT:
        _BUILT[uniform] = _build_bass_v3() if uniform else _build_bass_fallback()
    return _BUILT[uniform]


def kernel(**inputs) -> np.ndarray:
    host, uniform = _prep_host(inputs)
    nc = _get_built(uniform)
    res = bass_utils.run_bass_kernel_spmd(nc, [host], core_ids=[0], **RUN_KWARGS)
    kernel.last_result = res
    out_dev = res.results[0]["out"]                               # [T, C, 128]
    out = out_dev.reshape(T, C, BS, D).transpose(2, 0, 1, 3)      # [B,T,C,D]
    return np.ascontiguousarray(out)


if __name__ == "__main__":
    print("standalone smoke: building bass module (uniform decay path)...")
    _get_built(True)
    print("built ok")


# revision 18
# speedup vs baseline: 1.2476x; 1.0934x over previous
"""Trainium2 Bass kernel for nn_MemoryGraphBackprop (GNN message passing).

Strategy (v3)
-------------
T=64 sequential steps over state [BS=2, N=1024, D=64] on ONE NeuronCore
(the recurrence is latency-bound; an 8-core shard would need a per-step
all-gather that dwarfs the compute).  Everything SBUF-resident.

Math per step t:
    r   = A @ pm  (+ cc_t into nodes < C)
    g_t = decay * (1 - eot[b,t])          # per-partition in layout-2
    h'  = g_t*h + (1-g_t)*r               # h-state (NOT u=prim*h)
    pm' = tanh(prim * h')

v3 structure:
  - fp8(e4m3) DoubleRow matmuls: A and pm quantized to fp8; each matmul
    instruction contracts 2 src chunks (K=256) at 2x bf16 FLOP rate.
    Layout-2 psum r: [128 part = b*64+d, dst node free], 2 halves of 512.
  - h-state chain eliminates the per-step w2=(1-dt)*prim tensor:
      sb  = g_t*h (+ host-precomputed (1-g)*cc for nodes < C)   [DVE]
      h'  = (ps * (1-g_t)) + sb   -- one fused scalar_tensor_tensor/half
      u'  = h' * prim             -- half0 on GPSIMD, half1 on DVE
    then PE transposes u' quarters to layout-1, ACT fuses tanh into the
    PSUM->SBUF copy producing fp8 pm' (next step's stationary) and the
    fp32 output slice.
  - cc inject, eot gates all host-precomputed into [128,T]-indexed consts,
    so the device code is branch-free and input-agnostic (uniform decay).

Layouts:
  l2 (state h, psum r):  [128 part = b*64+d, 1024 free = n]
  l1 (pm, matmul lhsT):  [128 part = n%128, free = (n//128, b*64+d)]
"""

import os
import sys

if "/opt/trn_rl_repo" not in sys.path:
    sys.path.insert(0, "/opt/trn_rl_repo")

import numpy as np

import concourse.bass as bass
import concourse.mybir as mybir
import concourse.tile as tile
from concourse import bass_utils

BS, T, C, D = 2, 64, 64, 64
N = 1024
NT = N // 128  # 8 node chunks
P = 128        # BS*D partitions in layout-2

F32 = mybir.dt.float32
BF16 = mybir.dt.bfloat16
F8 = mybir.dt.float8e4
DR = mybir.MatmulPerfMode.DoubleRow
MULT = mybir.AluOpType.mult
ADD = mybir.AluOpType.add

# ---------------------------------------------------------------------------
# Workaround: this container's walrus accepts only ONE sync-wait per
# instruction.  (1) Tile's tail drain attaches one wait per live semaphore —
# split across multiple drains.  (2) Any multi-wait instruction gets its
# extra waits hoisted onto InstEventSemaphore carriers just before it.
# ---------------------------------------------------------------------------
from concourse.vector_clock import ScopedClock  # noqa: E402

SIM_MODE = False  # True: skip walrus-only workarounds so CoreSim can run
_ORIG_DRAIN = tile.TileContext._drain_and_barrier


def _patched_drain_and_barrier(self, tick_clock, wait_clock):
    if SIM_MODE:
        return _ORIG_DRAIN(self, tick_clock, wait_clock)
    drain_inst = self.nc.sync.drain()
    wait_clock.add_sem_waits(
        drain_inst.ins, ScopedClock({None: tick_clock.global_clock})
    )
    si = drain_inst.ins.sync_info
    if si is not None and si.on_wait is not None and len(si.on_wait) > 1:
        waits = list(si.on_wait)
        drain_inst.ins.sync_info = mybir.SyncInfo(
            on_wait=[waits[0]], on_update=si.on_update
        )
        for w in waits[1:]:
            d2 = self.nc.sync.drain()
            d2.ins.sync_info = mybir.SyncInfo(on_wait=[w], on_update=[])

    self.nc.all_engine_barrier()
    assert self.sems is not None
    popped = self.nc._tile_sem_poison_stack.pop()
    assert popped is self._sem_poison
    self.nc.clear_and_free_semaphores(list(self.sems.allocated().values()))
    self.nc.all_engine_barrier()


tile.TileContext._drain_and_barrier = _patched_drain_and_barrier


def _split_multi_waits(nc):
    if SIM_MODE:
        return 0
    n_carriers = 0
    for bb in nc.m.functions[0].blocks:
        insts = list(bb.instructions)
        out = []
        changed = False
        for inst in insts:
            si = inst.sync_info
            if si is not None and si.on_wait is not None and len(si.on_wait) > 1:
                waits = list(si.on_wait)
                for w in waits[:-1]:
                    n_carriers += 1
                    carrier = mybir.InstEventSemaphore(
                        name=f"waitsplit-{n_carriers}", ins=[], outs=[]
                    )
                    carrier.engine = inst.engine
                    carrier.sync_info = mybir.SyncInfo(on_wait=[w], on_update=[])
                    out.append(carrier)
                inst.sync_info = mybir.SyncInfo(
                    on_wait=[waits[-1]], on_update=si.on_update
                )
                changed = True
            out.append(inst)
        if changed:
            bb.instructions = out
    return n_carriers


# ---------------------------------------------------------------------------
# Host-side input massaging (layouts, scatter into dense A, norms, sigmoid).
# ---------------------------------------------------------------------------
def _prep_host(inputs):
    import ml_dtypes

    bf16 = ml_dtypes.bfloat16
    f8 = ml_dtypes.float8_e4m3

    cc = np.asarray(inputs["cc_signals"], dtype=np.float32)       # [B,T,C,D]
    eot = np.asarray(inputs["eot_mask"]).astype(bool)             # [B,T]
    idx = np.asarray(inputs["conn_indices"]).astype(np.int64)     # [N,K]
    cmask = np.asarray(inputs["conn_mask"]).astype(np.float32)    # [N,K]
    prim = np.asarray(inputs["primitives"], dtype=np.float32)     # [N,D]
    w = np.asarray(inputs["conn_weights"], dtype=np.float32)      # [N,K]
    dlog = np.asarray(inputs["decay_logit"], dtype=np.float32)    # [N]
    h0 = np.asarray(inputs["h0"], dtype=np.float32)               # [B,N,D]
    pm0 = np.asarray(inputs["prev_msg0"], dtype=np.float32)       # [B,N,D]

    # dense adjacency, transposed for the layout-2 matmul (rhs[m, n] = A[n, m])
    A = np.zeros((N, N), dtype=np.float32)
    np.add.at(A, (np.arange(N)[:, None], idx), w * cmask)
    At = np.ascontiguousarray(A.T)                                # [m, n]
    at_host = At.reshape(NT, 128, N).transpose(1, 0, 2).reshape(128, NT * N)

    # L2-normalized cc, layout-2: [b*64+d partitions, t*64 + n(<C) free]
    nrm = np.maximum(np.linalg.norm(cc, axis=-1, keepdims=True), 1e-8)
    ccn = (cc / nrm).astype(np.float32)
    cc2_host = np.ascontiguousarray(
        ccn.transpose(0, 3, 1, 2).reshape(P, T * C)
    )

    decay = (1.0 / (1.0 + np.exp(-dlog.astype(np.float64)))).astype(np.float32)
    uniform = bool(np.all(decay == decay[0]))

    prim_l2 = np.ascontiguousarray(np.tile(prim.T, (BS, 1)))      # [128, N]
    fmat = np.repeat((~eot).astype(np.float32), D, axis=0)        # [128, T]

    h0_l2 = h0.transpose(0, 2, 1).reshape(P, N)                   # [b*64+d, n]

    pm0_l1 = np.ascontiguousarray(
        pm0.reshape(BS, NT, 128, D).transpose(2, 1, 0, 3).reshape(128, NT * P)
    )

    host = {
        "at": at_host.astype(f8),
        "prim": prim_l2.astype(bf16),
        "pm0": pm0_l1.astype(f8),
    }
    if uniform:
        g = decay[0] * fmat                                       # [128, T]
        h1g = 1.0 - g
        host["gmat"] = np.ascontiguousarray(g.astype(np.float32))
        host["h1g"] = np.ascontiguousarray(h1g.astype(np.float32))
        host["h0"] = np.ascontiguousarray(h0_l2.astype(bf16))
        # (1-g)-scaled cc inject, indexed [p, t*C + c]
        ccw = cc2_host * np.repeat(h1g, C, axis=1)
        host["ccw"] = np.ascontiguousarray(ccw.astype(bf16))
    else:
        u0 = np.ascontiguousarray(prim_l2 * h0_l2)
        host["u0"] = u0.astype(bf16)
        host["cc2"] = cc2_host.astype(bf16)
        dec_l2 = np.ascontiguousarray(np.broadcast_to(decay[None, :], (P, N)))
        host["dec"] = dec_l2.astype(bf16)
        host["dp"] = (prim_l2 * decay[None, :]).astype(bf16)
        host["fmat"] = np.ascontiguousarray(fmat.astype(np.float32))
    return host, uniform


# ---------------------------------------------------------------------------
# Device kernel (v3, uniform-decay fast path)
# ---------------------------------------------------------------------------
def _build_bass_v3():
    nc = bass.Bass("TRN2", target_bir_lowering=False, debug=False)

    at_d = nc.dram_tensor("at", [128, NT * N], F8, kind="ExternalInput")
    prim_d = nc.dram_tensor("prim", [P, N], BF16, kind="ExternalInput")
    h0_d = nc.dram_tensor("h0", [P, N], BF16, kind="ExternalInput")
    pm0_d = nc.dram_tensor("pm0", [128, NT * P], F8, kind="ExternalInput")
    g_d = nc.dram_tensor("gmat", [P, T], F32, kind="ExternalInput")
    h1g_d = nc.dram_tensor("h1g", [P, T], F32, kind="ExternalInput")
    ccw_d = nc.dram_tensor("ccw", [P, T * C], BF16, kind="ExternalInput")
    out_d = nc.dram_tensor("out", [T, C, P], F32, kind="ExternalOutput")

    Tanh = mybir.ActivationFunctionType.Tanh
    STEPS = int(os.environ.get("KSTEPS", T))

    with tile.TileContext(nc) as tc:
        with (
            tc.tile_pool(name="consts", bufs=1) as consts,
            tc.tile_pool(name="state", bufs=3) as state,
            tc.tile_pool(name="tmp", bufs=3) as tmp,
            tc.tile_pool(name="psr", bufs=3, space="PSUM") as psr,
            tc.tile_pool(name="ptp", bufs=2, space="PSUM") as ptp,
        ):
            id128_sb = consts.tile([128, 128], BF16)
            from concourse.masks import make_identity
            make_identity(nc, id128_sb[:])

            # HAM warm-up: dummy matmuls keep the PE activity monitor at
            # full clock while the input DMAs land.
            warm_ps = psr.tile([128, 128], F32, tag="ps0", name="warm_ps")
            for i in range(64):
                nc.tensor.matmul(
                    warm_ps[:], id128_sb[:], id128_sb[:],
                    start=(i == 0), stop=(i == 63), skip_group_check=True,
                )

            # --- state + consts loads (state first so step-0 deps clear) ---
            h = state.tile([P, N], BF16, tag="h", name="h")
            pm = [
                state.tile([128, 4, 128], F8, tag="pm0h", name="pm_lo"),
                state.tile([128, 4, 128], F8, tag="pm1h", name="pm_hi"),
            ]
            nc.sync.dma_start(out=h[:], in_=h0_d.ap()[:])
            for hh in range(2):
                nc.sync.dma_start(
                    out=pm[hh][:], in_=pm0_d.ap()[:, hh * 512:(hh + 1) * 512]
                )
            at_sb = consts.tile([128, NT, N], F8)
            for hh in range(2):
                for m in range(NT):
                    nc.sync.dma_start(
                        out=at_sb[:, m, hh * 512:(hh + 1) * 512],
                        in_=at_d.ap()[:, m * N + hh * 512:m * N + (hh + 1) * 512],
                    )
            prim_sb = consts.tile([P, N], BF16)
            nc.sync.dma_start(out=prim_sb[:], in_=prim_d.ap()[:])
            g_sb = consts.tile([P, T], F32)
            nc.sync.dma_start(out=g_sb[:], in_=g_d.ap()[:])
            h1g_sb = consts.tile([P, T], F32)
            nc.sync.dma_start(out=h1g_sb[:], in_=h1g_d.ap()[:])
            ccw_sb = consts.tile([P, T * C], BF16)
            for q in range(4):
                s = slice(q * (T * C) // 4, (q + 1) * (T * C) // 4)
                nc.sync.dma_start(out=ccw_sb[:, s], in_=ccw_d.ap()[:, s])

            for t in range(STEPS):
                last = t == T - 1
                gt = g_sb[:, t:t + 1]
                ngt = h1g_sb[:, t:t + 1]

                # ---- sb = g*h (+ (1-g)*cc on nodes < C); off critical path
                sb = tmp.tile([P, N], BF16, tag="sb")
                nc.vector.scalar_tensor_tensor(
                    sb[:, 0:C], h[:, 0:C], gt,
                    ccw_sb[:, t * C:(t + 1) * C], MULT, ADD,
                )
                nc.vector.tensor_scalar_mul(sb[:, C:512], h[:, C:512], gt)
                if not last:
                    nc.vector.tensor_scalar_mul(sb[:, 512:N], h[:, 512:N], gt)

                # ---- matmuls: r = A @ pm (fp8 DoubleRow), layout-2 psum ----
                ps = [
                    psr.tile([P, 512], F32, tag="ps0", name="ps0"),
                    psr.tile([P, 512], F32, tag="ps1", name="ps1"),
                ]
                for hh in range(2):
                    if last and hh == 1:
                        break  # last step: only chunk 0 reaches the output
                    for jp in range(4):  # src-chunk pairs (2jp, 2jp+1)
                        nc.tensor.matmul(
                            ps[hh][:],
                            pm[jp // 2][:, (jp % 2) * 2:(jp % 2) * 2 + 2, :],
                            at_sb[:, 2 * jp:2 * jp + 2, hh * 512:(hh + 1) * 512],
                            start=(jp == 0),
                            stop=(jp == 3),
                            perf_mode=DR,
                            skip_group_check=True,
                        )

                # ---- chain: h' = ps*(1-g) + sb ; u' = h'*prim ----
                hn = state.tile([P, N], BF16, tag="h", name="hn")
                un = tmp.tile([P, N], BF16, tag="un")
                pmn = [
                    state.tile([128, 4, 128], F8, tag="pm0h", name="pmn_lo"),
                    state.tile([128, 4, 128], F8, tag="pm1h", name="pmn_hi"),
                ]
                pts = [
                    ptp.tile([128, 256], BF16, tag="pt", name="pt")
                    for _ in range(4)
                ]
                out_sb = tmp.tile([C, P], F32, tag="out_sb")

                if last:
                    nc.vector.scalar_tensor_tensor(
                        hn[:, 0:128], ps[0][:, 0:128], ngt, sb[:, 0:128],
                        MULT, ADD,
                    )
                    nc.vector.tensor_mul(
                        un[:, 0:128], hn[:, 0:128], prim_sb[:, 0:128]
                    )
                    nc.tensor.transpose(
                        pts[0][:, 0:128], un[:, 0:128], id128_sb[:]
                    )
                    nc.scalar.activation(out_sb[:], pts[0][0:C, 0:P], Tanh)
                    nc.sync.dma_start(out=out_d.ap()[t], in_=out_sb[:])
                    break

                for q in range(4):
                    qsl = slice(q * 256, (q + 1) * 256)
                    psl = slice((q % 2) * 256, (q % 2 + 1) * 256)
                    # quarter-granular chain: STT from psum, then prim-mult,
                    # both on DVE (in-order, no extra sem between them).
                    nc.vector.scalar_tensor_tensor(
                        hn[:, qsl], ps[q // 2][:, psl], ngt, sb[:, qsl],
                        MULT, ADD,
                    )
                    nc.vector.tensor_mul(
                        un[:, qsl], hn[:, qsl], prim_sb[:, qsl]
                    )
                    for j in range(2):
                        ch = q * 2 + j
                        nc.tensor.transpose(
                            pts[q][:, j * 128:(j + 1) * 128],
                            un[:, ch * 128:(ch + 1) * 128],
                            id128_sb[:],
                        )
                    nc.scalar.activation(
                        pmn[q // 2][:, (q % 2) * 2:(q % 2) * 2 + 2, :],
                        pts[q][:], Tanh,
                    )

                # fp32 output slice (nodes < C live in transposed chunk 0)
                nc.scalar.activation(out_sb[:], pts[0][0:C, 0:P], Tanh)
                nc.sync.dma_start(out=out_d.ap()[t], in_=out_sb[:])

                h, pm = hn, pmn

    _split_multi_waits(nc)
    return nc


# ---------------------------------------------------------------------------
# Fallback kernel (non-uniform decay): previous-generation structure.
# ---------------------------------------------------------------------------
def _build_bass_fallback():
    nc = bass.Bass("TRN2", target_bir_lowering=False, debug=False)

    at_d = nc.dram_tensor("at", [128, NT * N], F8, kind="ExternalInput")
    cc2_d = nc.dram_tensor("cc2", [P, T * C], BF16, kind="ExternalInput")
    prim_d = nc.dram_tensor("prim", [P, N], BF16, kind="ExternalInput")
    u0_d = nc.dram_tensor("u0", [P, N], BF16, kind="ExternalInput")
    pm0_d = nc.dram_tensor("pm0", [128, NT * P], F8, kind="ExternalInput")
    out_d = nc.dram_tensor("out", [T, C, P], F32, kind="ExternalOutput")
    dec_d = nc.dram_tensor("dec", [P, N], BF16, kind="ExternalInput")
    dp_d = nc.dram_tensor("dp", [P, N], BF16, kind="ExternalInput")
    f_d = nc.dram_tensor("fmat", [P, T], F32, kind="ExternalInput")

    Tanh = mybir.ActivationFunctionType.Tanh

    with tile.TileContext(nc) as tc:
        with (
            tc.tile_pool(name="consts", bufs=1) as consts,
            tc.tile_pool(name="state", bufs=3) as state,
            tc.tile_pool(name="tmp", bufs=3) as tmp,
            tc.tile_pool(name="psr", bufs=3, space="PSUM") as psr,
            tc.tile_pool(name="ptp", bufs=2, space="PSUM") as ptp,
        ):
            id128_sb = consts.tile([128, 128], BF16)
            from concourse.masks import make_identity
            make_identity(nc, id128_sb[:])

            warm_ps = psr.tile([128, 128], F32, tag="ps0", name="warm_ps")
            for i in range(64):
                nc.tensor.matmul(
                    warm_ps[:], id128_sb[:], id128_sb[:],
                    start=(i == 0), stop=(i == 63), skip_group_check=True,
                )

            u = [
                state.tile([P, 512], BF16, tag="u0h", name="u_lo"),
                state.tile([P, 512], BF16, tag="u1h", name="u_hi"),
            ]
            pm = [
                state.tile([128, 4, 128], F8, tag="pm0h", name="pm_lo"),
                state.tile([128, 4, 128], F8, tag="pm1h", name="pm_hi"),
            ]
            for hh in range(2):
                nc.sync.dma_start(out=u[hh][:], in_=u0_d.ap()[:, hh * 512:(hh + 1) * 512])
                nc.sync.dma_start(out=pm[hh][:], in_=pm0_d.ap()[:, hh * 512:(hh + 1) * 512])
            at_sb = consts.tile([128, NT, N], F8)
            for hh in range(2):
                for m in range(NT):
                    nc.sync.dma_start(
                        out=at_sb[:, m, hh * 512:(hh + 1) * 512],
                        in_=at_d.ap()[:, m * N + hh * 512:m * N + (hh + 1) * 512],
                    )
            prim_sb = consts.tile([P, N], BF16)
            nc.sync.dma_start(out=prim_sb[:], in_=prim_d.ap()[:])
            dec_sb = consts.tile([P, N], BF16)
            nc.sync.dma_start(out=dec_sb[:], in_=dec_d.ap()[:])
            dp_sb = consts.tile([P, N], BF16)
            nc.sync.dma_start(out=dp_sb[:], in_=dp_d.ap()[:])
            f_sb = consts.tile([P, T], F32)
            nc.sync.dma_start(out=f_sb[:], in_=f_d.ap()[:])
            cc2_sb = consts.tile([P, T * C], BF16)
            for q in range(4):
                s = slice(q * (T * C) // 4, (q + 1) * (T * C) // 4)
                nc.sync.dma_start(out=cc2_sb[:, s], in_=cc2_d.ap()[:, s])

            for t in range(T):
                sb_t = [
                    tmp.tile([P, 512], BF16, tag="sb0", name="sb_lo"),
                    tmp.tile([P, 512], BF16, tag="sb1", name="sb_hi"),
                ]
                w2 = tmp.tile([P, N], BF16, tag="w2")
                ft = f_sb[:, t:t + 1]
                w0 = tmp.tile([P, N], BF16, tag="w0")
                nc.vector.tensor_scalar_mul(w0[:], dec_sb[:], ft)
                nc.vector.tensor_mul(sb_t[0][:], u[0][:], w0[:, 0:512])
                nc.vector.tensor_mul(sb_t[1][:], u[1][:], w0[:, 512:1024])
                w1 = tmp.tile([P, N], BF16, tag="w1")
                nc.vector.tensor_scalar_mul(w1[:], dp_sb[:], ft)
                nc.vector.tensor_sub(w2[:], prim_sb[:], w1[:])
                cw = tmp.tile([P, C], BF16, tag="cw")
                nc.vector.tensor_mul(
                    cw[:], w2[:, 0:C], cc2_sb[:, t * C:(t + 1) * C]
                )
                nc.vector.tensor_add(sb_t[0][:, 0:C], sb_t[0][:, 0:C], cw[:])

                ps = [
                    psr.tile([P, 512], F32, tag="ps0", name="ps0"),
                    psr.tile([P, 512], F32, tag="ps1", name="ps1"),
                ]
                un = [
                    state.tile([P, 512], BF16, tag="u0h", name="un_lo"),
                    state.tile([P, 512], BF16, tag="u1h", name="un_hi"),
                ]
                pmn = [
                    state.tile([128, 4, 128], F8, tag="pm0h", name="pmn_lo"),
                    state.tile([128, 4, 128], F8, tag="pm1h", name="pmn_hi"),
                ]
                pts = [
                    ptp.tile([128, 256], BF16, tag="pt", name="pt")
                    for _ in range(4)
                ]
                for hh in range(2):
                    for jp in range(4):
                        nc.tensor.matmul(
                            ps[hh][:],
                            pm[jp // 2][:, (jp % 2) * 2:(jp % 2) * 2 + 2, :],
                            at_sb[:, 2 * jp:2 * jp + 2, hh * 512:(hh + 1) * 512],
                            start=(jp == 0),
                            stop=(jp == 3),
                            perf_mode=DR,
                            skip_group_check=True,
                        )
                    for hq in range(2):
                        q = hh * 2 + hq
                        if t == T - 1 and q > 0:
                            continue
                        psl = slice(hq * 256, (hq + 1) * 256)
                        x = tmp.tile([P, 256], BF16, tag=f"x{q}", name="x")
                        nc.vector.tensor_mul(
                            x[:], ps[hh][:, psl], w2[:, q * 256:(q + 1) * 256]
                        )
                        nc.vector.tensor_add(
                            un[hh][:, psl], x[:], sb_t[hh][:, psl]
                        )

                out_sb = tmp.tile([C, P], F32, tag="out_sb")
                for q in range(4):
                    if t == T - 1 and q > 0:
                        continue
                    hh, hq = divmod(q, 2)
                    for j in range(2):
                        if t == T - 1 and (hq * 2 + j) > 0:
                            continue
                        mloc = hq * 2 + j
                        nc.tensor.transpose(
                            pts[q][:, j * 128:(j + 1) * 128],
                            un[hh][:, mloc * 128:(mloc + 1) * 128],
                            id128_sb[:],
                        )
                    if t < T - 1:
                        nc.scalar.activation(
                            pmn[q // 2][:, (q % 2) * 2:(q % 2) * 2 + 2, :],
                            pts[q][:], Tanh,
                        )
                    if q == 0:
                        nc.scalar.activation(out_sb[:], pts[0][0:C, 0:P], Tanh)
                nc.sync.dma_start(out=out_d.ap()[t], in_=out_sb[:])

                u, pm = un, pmn

    _split_multi_waits(nc)
    return nc


RUN_KWARGS: dict = {}
_BUILT: dict = {}


def _get_built(uniform):
    if uniform not in _BUILT:
        _BUILT[uniform] = _build_bass_v3() if uniform else _build_bass_fallback()
    return _BUILT[uniform]


def kernel(**inputs) -> np.ndarray:
    host, uniform = _prep_host(inputs)
    nc = _get_built(uniform)
    res = bass_utils.run_bass_kernel_spmd(nc, [host], core_ids=[0], **RUN_KWARGS)
    kernel.last_result = res
    out_dev = res.results[0]["out"]                               # [T, C, 128]
    out = out_dev.reshape(T, C, BS, D).transpose(2, 0, 1, 3)      # [B,T,C,D]
    return np.ascontiguousarray(out)


if __name__ == "__main__":
    print("standalone smoke: building bass module (uniform decay path)...")
    _get_built(True)
    print("built ok")
